# revision 2
# baseline (speedup 1.0000x reference)
"""Multi-head attention (B=8, N=1024, C=1024, H=16, D=64) with QK RMS-norm,
data-parallel across 8 NeuronCores (one batch element per core).

v4: engines execute their instruction streams in order, so overlap is baked
into emission order. One software-pipelined loop over head pairs; each
pair's 16 (nh, st) attention slots carry the S^T matmul pair (lookahead-1),
the exp, the AV accumulation, plus "filler" chunks popped from a queue
(next pair's q/k projection + RMS chain, V projection tiles, previous
pair's denominator broadcast, tail projection chains). Fillers are ordered
so PE-independent work covers every cross-engine latency chain — PE never
idles >2us, keeping the HAM clock gate at full rate.

Inputs are host-relayouted to partition-major so every DMA line is >=2KB
contiguous. All matmul inputs bf16 (fp32 PSUM accumulation). RMS rsqrt is
computed on the DVE (bit-trick + 2 Newton steps) so ScalarE only ever
loads the exp table set.
"""

import numpy as np

import concourse.bacc as bacc
import concourse.bass as bass
import concourse.tile as tile
from concourse import mybir
from concourse.bass_utils import run_bass_kernel_spmd

F32 = mybir.dt.float32
F32R = mybir.dt.float32r
BF16 = mybir.dt.bfloat16
I32 = mybir.dt.int32
AF = mybir.ActivationFunctionType
OP = mybir.AluOpType

B, N, C = 8, 1024, 1024
H, D = 16, 64
EPS = 1e-6
NCORES = 8
NT = N // 128
CT = C // 128
NPAIR = H // 2
MAGIC = 0x5F3759DF

SLOT_BUDGET = 1000


def _build():
    nc = bacc.Bacc(None, target_bir_lowering=False)

    xP_d = nc.dram_tensor("xP", [128, CT, N], BF16, kind="ExternalInput")
    wqp_d = nc.dram_tensor("wqP", [2 * NPAIR, 128, CT, 128], BF16, kind="ExternalInput")
    wvp_d = nc.dram_tensor("wvP", [128, CT, C], BF16, kind="ExternalInput")
    wpp_d = nc.dram_tensor("wpP", [128, CT, C], BF16, kind="ExternalInput")
    bqkv_d = nc.dram_tensor("bqkv", [3 * C], F32, kind="ExternalInput")
    bproj_d = nc.dram_tensor("bproj", [C], F32, kind="ExternalInput")
    selq_d = nc.dram_tensor("selq", [8, 128], F32, kind="ExternalInput")
    selk_d = nc.dram_tensor("selk", [8, 128], F32, kind="ExternalInput")
    y_d = nc.dram_tensor("y", [N, C], BF16, kind="ExternalOutput")

    with tile.TileContext(nc) as tc:
        with (
            tc.tile_pool(name="long", bufs=1) as lp,
            tc.tile_pool(name="wq", bufs=3) as wq,
            tc.tile_pool(name="qkp", bufs=6) as qkp,
            tc.tile_pool(name="sqp", bufs=2) as sqp,
            tc.tile_pool(name="rpp", bufs=4) as rpp,
            tc.tile_pool(name="rrw", bufs=4) as rrw,
            tc.tile_pool(name="pbp", bufs=4) as pbp,
            tc.tile_pool(name="stp", bufs=4) as stp,
            tc.tile_pool(name="ysp", bufs=3) as ysp,
            tc.tile_pool(name="pp", bufs=2, space="PSUM") as pp,
            tc.tile_pool(name="sps", bufs=2, space="PSUM") as sps,
            tc.tile_pool(name="oo", bufs=1, space="PSUM") as oo,
        ):
            # ---------------- persistent tiles ----------------
            xt = lp.tile([128, CT, N], BF16, tag="xt", name="xt")
            vaug = [lp.tile([128, H, D + 1], BF16, tag=f"va{i}", name=f"va{i}")
                    for i in range(NT)]
            attnT = [lp.tile([128, N], BF16, tag=f"at{i}", name=f"at{i}")
                     for i in range(NPAIR)]
            wv = lp.tile([128, CT, C], BF16, tag="wv", name="wv")
            wpj = lp.tile([128, CT, C], BF16, tag="wpj", name="wpj")
            selq = lp.tile([98, 128], F32R, tag="selq", name="selq")
            selk = lp.tile([98, 128], F32R, tag="selk", name="selk")
            rq_sb = [lp.tile([98, N], F32R, tag=f"rqs{g}", name=f"rqs{g}") for g in range(2)]
            rk_sb = [lp.tile([98, N], F32R, tag=f"rks{g}", name=f"rks{g}") for g in range(2)]
            dv_sb = [lp.tile([98, N], F32R, tag=f"dvs{g}", name=f"dvs{g}") for g in range(2)]
            ones2 = lp.tile([128, 2], BF16, tag="ones2", name="ones2")
            ones1 = lp.tile([1, 128], F32R, tag="ones1", name="ones1")
            magic = lp.tile([128, 32], I32, tag="magic", name="magic")
            bv_bc = lp.tile([128, C], BF16, tag="bvbc", name="bvbc")
            bp_bc = lp.tile([128, C], F32, tag="bpbc", name="bpbc")

            # ---------------- prologue DMAs + constants ----------------
            def load_jt_dma(jt):
                bias_c = wq.tile([128, 1], F32, tag="biasc", name=f"bc{jt}")
                nc.sync.dma_start(out=bias_c,
                                  in_=bqkv_d[jt * 128:(jt + 1) * 128].unsqueeze(1))
                wts = wq.tile([128, CT, 128], BF16, tag="wts", name=f"wts{jt}")
                nc.sync.dma_start(out=wts, in_=wqp_d[jt, :, :, :])
                return bias_c, wts

            # DMA queue drains in emission order: order strictly by first need.
            pre0 = load_jt_dma(0)
            for ct in range(4):
                nc.sync.dma_start(out=xt[:, ct:ct + 1, :], in_=xP_d[:, ct:ct + 1, :])
            pre8 = load_jt_dma(NPAIR)
            for ct in range(4, CT):
                nc.sync.dma_start(out=xt[:, ct:ct + 1, :], in_=xP_d[:, ct:ct + 1, :])
            browv = wq.tile([1, C], F32R, tag="brow", name="browv")
            nc.sync.dma_start(out=browv, in_=bqkv_d[2 * C:3 * C].unsqueeze(0).bitcast(F32R))
            browp = wq.tile([1, C], F32R, tag="brow", name="browp")
            nc.sync.dma_start(out=browp, in_=bproj_d[:].unsqueeze(0).bitcast(F32R))
            nc.sync.dma_start(out=wv[:, :, 0:512], in_=wvp_d[:, :, 0:512])
            for j in range(4):
                nc.sync.dma_start(out=selq[32 * j:32 * j + 2, :],
                                  in_=selq_d[2 * j:2 * j + 2, :].bitcast(F32R))
                nc.sync.dma_start(out=selk[32 * j:32 * j + 2, :],
                                  in_=selk_d[2 * j:2 * j + 2, :].bitcast(F32R))
            nc.sync.dma_start(out=wv[:, :, 512:1024], in_=wvp_d[:, :, 512:1024])

            nc.vector.memset(ones2, 0.0)
            nc.vector.memset(ones2[0:64, 0:1], 1.0)
            nc.vector.memset(ones2[64:128, 1:2], 1.0)
            nc.vector.memset(ones1.bitcast(F32), 1.0)
            nc.vector.memset(magic, MAGIC)
            for ntx in range(NT):
                nc.vector.memset(vaug[ntx][:, :, D:D + 1], 1.0)

            def bias_bc_mms():
                for (brow, btile) in ((browv, bv_bc), (browp, bp_bc)):
                    for half in range(2):
                        hs = slice(half * 512, (half + 1) * 512)
                        bb = pp.tile([128, 512], F32, tag="pp", name=f"bb{half}")
                        nc.tensor.matmul(out=bb[:, :], lhsT=ones1[:, :],
                                         rhs=brow[:, hs], start=True, stop=True)
                        nc.vector.tensor_copy(out=btile[:, hs], in_=bb[:, :])

            # ---------------- chunked helpers ----------------
            def qk_jtile_chunks(jt, qk, sq, rp2, preload=None):
                """3 chunks: [dma + interleaved ct0-3] [ct4-7 + evac/sq] [ssq+rp2].
                nh0/nh1 chains interleave so consecutive matmuls share lhsT."""
                state = {}

                def c_mm(lo):
                    def f():
                        if lo == 0:
                            bias_c, wts = preload if preload else load_jt_dma(jt)
                            state["bias"], state["wts"] = bias_c, wts
                            state["ps0"] = pp.tile([128, 512], F32, tag="pp",
                                                   name=f"qp{jt}0")
                            state["ps1"] = pp.tile([128, 512], F32, tag="pp",
                                                   name=f"qp{jt}1")
                        for ct in range(lo, lo + 4):
                            for nh in range(2):
                                nc.tensor.matmul(
                                    out=state[f"ps{nh}"][:, :],
                                    lhsT=state["wts"][:, ct, :],
                                    rhs=xt[:, ct, nh * 512:(nh + 1) * 512],
                                    start=(ct == 0), stop=(ct == CT - 1),
                                )
                        if lo == 4:
                            for nh in range(2):
                                nsl = slice(nh * 512, (nh + 1) * 512)
                                nc.vector.tensor_scalar(
                                    out=qk[:, nsl], in0=state[f"ps{nh}"][:, :],
                                    scalar1=state["bias"][:, :], scalar2=None, op0=OP.add)
                                nc.vector.tensor_mul(sq[:, nsl], qk[:, nsl], qk[:, nsl])
                    return f

                def c_ssq():
                    def f():
                        for nh in range(2):
                            nsl = slice(nh * 512, (nh + 1) * 512)
                            psum = state[f"ps{nh}"]
                            nc.tensor.matmul(out=psum[0:2, :], lhsT=ones2[:, :],
                                             rhs=sq[:, nsl], start=True, stop=True)
                            nc.vector.tensor_scalar(out=rp2[:, nsl], in0=psum[0:2, :],
                                                    scalar1=1.0 / D, scalar2=EPS,
                                                    op0=OP.mult, op1=OP.add)
                    return f

                return [(1800, c_mm(0)), (1800, c_mm(4)), (500, c_ssq())]

            def rms_chunk(p, rp2q, rp2k):
                def f():
                    g, j = divmod(p, 4)
                    rr = rrw.tile([128, 32], F32, tag="rr", name=f"rr{p}")
                    t0 = rrw.tile([128, 32], I32, tag="t0", name=f"t0{p}")
                    t1 = rrw.tile([128, 32], F32, tag="t1", name=f"t1{p}")
                    nc.sync.dma_start(out=rr[0:64, :], in_=rp2q[:, :])
                    nc.sync.dma_start(out=rr[64:128, :], in_=rp2k[:, :])
                    vi = rr.bitcast(I32)
                    nc.vector.tensor_scalar(out=t0, in0=vi, scalar1=1, scalar2=None,
                                            op0=OP.logical_shift_right)
                    nc.vector.tensor_tensor(out=t0, in0=magic, in1=t0, op=OP.subtract)
                    y = t0.bitcast(F32)
                    for _ in range(2):
                        nc.vector.tensor_mul(t1, y, y)
                        nc.vector.tensor_mul(t1, t1, rr)
                        nc.vector.tensor_scalar(out=t1, in0=t1, scalar1=-0.5,
                                                scalar2=1.5, op0=OP.mult, op1=OP.add)
                        nc.vector.tensor_mul(y, y, t1)
                    nc.sync.dma_start(out=rq_sb[g][32 * j:32 * j + 2, :],
                                      in_=y[0:64, :].bitcast(F32R))
                    nc.sync.dma_start(out=rk_sb[g][32 * j:32 * j + 2, :],
                                      in_=y[64:128, :].bitcast(F32R))
                return [(100, f)]

            def scale_chunks(p, qk_q, qk_k):
                g, j = divmod(p, 4)
                out = []
                for (sel, rsb, qktile) in ((selq, rq_sb[g], qk_q), (selk, rk_sb[g], qk_k)):
                    for nh in range(2):
                        def f(sel=sel, rsb=rsb, qktile=qktile, nh=nh):
                            nsl = slice(nh * 512, (nh + 1) * 512)
                            bc = pp.tile([128, 512], F32, tag="pp", name=f"sc{p}{nh}")
                            nc.tensor.matmul(out=bc[:, :],
                                             lhsT=sel[32 * j:32 * j + 2, :],
                                             rhs=rsb[32 * j:32 * j + 2, nsl],
                                             start=True, stop=True,
                                             tile_position=(32 * j, 0))
                            nc.vector.tensor_tensor(out=qktile[:, nsl], in0=qktile[:, nsl],
                                                    in1=bc[:, :], op=OP.mult)
                        out.append((300, f))
                return out

            def v_chunk(nt, jh):
                def f():
                    vp = pp.tile([128, 512], F32, tag="pp", name=f"vp{nt}{jh}")
                    for ct in range(CT):
                        nc.tensor.matmul(
                            out=vp[:, :],
                            lhsT=xt[:, ct, nt * 128:(nt + 1) * 128],
                            rhs=wv[:, ct, jh * 512:(jh + 1) * 512],
                            start=(ct == 0), stop=(ct == CT - 1),
                        )
                    dst = vaug[nt][:, jh * 8:(jh + 1) * 8, 0:D]
                    vsrc = vp[:, :].rearrange("p (h d) -> p h d", d=D)
                    bsrc = bv_bc[:, jh * 512:(jh + 1) * 512].rearrange("p (h d) -> p h d", d=D)
                    nc.vector.tensor_tensor(out=dst, in0=vsrc, in1=bsrc, op=OP.add)
                return (1750, f)

            def dbp_chunk(p, dr):
                g, j = divmod(p, 4)

                def f():
                    nc.vector.reciprocal(out=dr, in_=dr)
                    nc.sync.dma_start(out=dv_sb[g][32 * j:32 * j + 2, :],
                                      in_=dr[:, :].bitcast(F32R))
                    dbp = sps.tile([128, 1024], F32, tag="sp", name=f"dbp{p}")
                    for nh in range(2):
                        nsl = slice(nh * 512, (nh + 1) * 512)
                        nc.tensor.matmul(out=dbp[:, nsl],
                                         lhsT=selq[32 * j:32 * j + 2, :],
                                         rhs=dv_sb[g][32 * j:32 * j + 2, nsl],
                                         start=True, stop=True, tile_position=(32 * j, 0))
                    nc.vector.tensor_tensor(out=attnT[p][:, :], in0=attnT[p][:, :],
                                            in1=dbp[:, :], op=OP.mult)
                return (500, f)

            def attention_pair(p, qk_q, qk_k, fillers):
                dr = rrw.tile([128, 16], F32, tag="dr", name=f"dr{p}")
                slots = [(nh, st) for nh in range(2) for st in range(NT)]
                psbs = {}

                def emit_S(nh, st):
                    nsl = slice(nh * 512, (nh + 1) * 512)
                    spt = sps.tile([128, 1024], F32, tag="sp", name=f"sp{p}{nh}{st}")
                    nc.tensor.matmul(
                        out=spt[:, 0:512],
                        lhsT=qk_k[0:64, st * 128:(st + 1) * 128],
                        rhs=qk_q[0:64, nsl],
                        start=True, stop=True, tile_position=(0, 0),
                    )
                    nc.tensor.matmul(
                        out=spt[:, 512:1024],
                        lhsT=qk_k[64:128, st * 128:(st + 1) * 128],
                        rhs=qk_q[64:128, nsl],
                        start=True, stop=True, tile_position=(64, 0),
                    )
                    psb = pbp.tile([128, 2, 512], BF16, tag="psb", name=f"pb{p}{nh}{st}")
                    nc.scalar.activation(
                        out=psb, in_=spt[:, :].rearrange("p (h n) -> p h n", h=2),
                        func=AF.Exp)
                    psbs[(nh, st)] = psb

                o_h = None
                for _ in range(2):
                    if fillers:
                        fillers.pop(0)[1]()
                emit_S(0, 0)
                for i, (nh, st) in enumerate(slots):
                    if st == 0:
                        o_h = oo.tile([D + 1, 2, 512], F32, tag="oo", name=f"o{p}{nh}")
                    budget = SLOT_BUDGET
                    while fillers and budget > 0:
                        cost, f = fillers.pop(0)
                        f()
                        budget -= cost
                    if i + 1 < len(slots):
                        emit_S(*slots[i + 1])
                    psb = psbs.pop((nh, st))
                    nc.tensor.matmul(
                        out=o_h[:, 0, :], lhsT=vaug[st][:, 2 * p, :], rhs=psb[:, 0, :],
                        start=(st == 0), stop=(st == NT - 1),
                    )
                    nc.tensor.matmul(
                        out=o_h[:, 1, :], lhsT=vaug[st][:, 2 * p + 1, :], rhs=psb[:, 1, :],
                        start=(st == 0), stop=(st == NT - 1),
                    )
                    if st == NT - 1:
                        nsl = slice(nh * 512, (nh + 1) * 512)
                        nc.vector.tensor_copy(out=attnT[p][0:64, nsl], in_=o_h[0:D, 0, :])
                        nc.vector.tensor_copy(out=attnT[p][64:128, nsl], in_=o_h[0:D, 1, :])
                        stg = stp.tile([1, 2, 512], F32, tag="stg", name=f"stg{p}{nh}")
                        nc.vector.tensor_copy(out=stg, in_=o_h[D:D + 1, :, :])
                        nc.sync.dma_start(out=dr[32 * nh:32 * nh + 32, :], in_=stg[:, 0, :])
                        nc.sync.dma_start(out=dr[64 + 32 * nh:96 + 32 * nh, :],
                                          in_=stg[:, 1, :])
                while fillers:
                    fillers.pop(0)[1]()
                return dr

            # ---------------- schedule ----------------
            def make_qk(jt):
                qk = qkp.tile([128, N], BF16, tag="qk", name=f"qk{jt}")
                sq = sqp.tile([128, N], BF16, tag="sq", name=f"sq{jt}")
                rp2 = rpp.tile([2, N], F32, tag="rp2", name=f"rp2{jt}")
                return qk, sq, rp2

            qk_q0, sq_q0, rp2q0 = make_qk(0)
            qk_k0, sq_k0, rp2k0 = make_qk(NPAIR)
            for _, f in qk_jtile_chunks(0, qk_q0, sq_q0, rp2q0, preload=pre0):
                f()
            for _, f in qk_jtile_chunks(NPAIR, qk_k0, sq_k0, rp2k0, preload=pre8):
                f()
            bias_bc_mms()
            v_chunk(0, 0)[1]()
            v_chunk(1, 0)[1]()
            rms_chunk(0, rp2q0, rp2k0)[0][1]()
            v_chunk(2, 0)[1]()
            v_chunk(3, 0)[1]()
            for _, f in scale_chunks(0, qk_q0, qk_k0):
                f()
            for nt in range(4, NT):
                v_chunk(nt, 0)[1]()
            qk_q1, sq_q1, rp2q1 = make_qk(1)
            qk_k1, sq_k1, rp2k1 = make_qk(NPAIR + 1)
            for _, f in (qk_jtile_chunks(1, qk_q1, sq_q1, rp2q1)
                         + qk_jtile_chunks(NPAIR + 1, qk_k1, sq_k1, rp2k1)
                         + rms_chunk(1, rp2q1, rp2k1)):
                f()

            # ---------------- output projection pieces ----------------
            def proj_chain_a(ch, nt, lo, hi):
                """accumulate ct in [lo, hi) into a held psum tile."""
                yp = pp.tile([128, 512], F32, tag="pp", name=f"yp{ch}{nt}")

                def f():
                    for ct in range(lo, hi):
                        nc.tensor.matmul(
                            out=yp[:, :],
                            lhsT=attnT[ct][:, nt * 128:(nt + 1) * 128],
                            rhs=wpj[:, ct, ch * 512:(ch + 1) * 512],
                            start=(ct == 0), stop=False,
                        )
                return yp, f

            def proj_mid(ch, nt, yp, lo, hi):
                for ct in range(lo, hi):
                    nc.tensor.matmul(
                        out=yp[:, :],
                        lhsT=attnT[ct][:, nt * 128:(nt + 1) * 128],
                        rhs=wpj[:, ct, ch * 512:(ch + 1) * 512],
                        start=False, stop=False,
                    )

            def proj_fin(ch, nt, yp):
                ct = CT - 1
                nc.tensor.matmul(
                    out=yp[:, :],
                    lhsT=attnT[ct][:, nt * 128:(nt + 1) * 128],
                    rhs=wpj[:, ct, ch * 512:(ch + 1) * 512],
                    start=False, stop=True,
                )
                ysb = ysp.tile([128, 512], BF16, tag="ysb", name=f"ysb{ch}{nt}")
                nc.vector.tensor_tensor(out=ysb, in0=yp[:, :],
                                        in1=bp_bc[:, ch * 512:(ch + 1) * 512], op=OP.add)
                nc.sync.dma_start(
                    out=y_d[nt * 128:(nt + 1) * 128, ch * 512:(ch + 1) * 512],
                    in_=ysb,
                )

            qk_tiles = {0: (qk_q0, qk_k0), 1: (qk_q1, qk_k1)}
            dbps = {}
            pre_chains = []
            for p in range(NPAIR):
                # scale for pair p+1 (its rsqrt finished a pair ago) leads the
                # queue: always-ready boundary bridges for the PE stream
                indep = []
                if p + 1 < NPAIR:
                    indep += scale_chunks(p + 1, *qk_tiles[p + 1])
                else:
                    for ci in range(2):
                        yp, f = proj_chain_a(ci, 0, 0, 5)
                        pre_chains.append((ci, 0, yp))
                        indep.append((1100, f))
                fillers = indep[:2]
                if p - 2 in dbps:
                    fillers.append(dbps.pop(p - 2))
                if p == NPAIR - 1:
                    fillers.append(dbps.pop(p - 1))
                fillers += indep[2:]
                if p + 2 < NPAIR:
                    jq, jk = p + 2, NPAIR + p + 2
                    qk_qn, sq_qn, rp2qn = make_qk(jq)
                    qk_kn, sq_kn, rp2kn = make_qk(jk)
                    qk_tiles[p + 2] = (qk_qn, qk_kn)
                    fillers += qk_jtile_chunks(jq, qk_qn, sq_qn, rp2qn)
                    fillers += qk_jtile_chunks(jk, qk_kn, sq_kn, rp2kn)
                    fillers += rms_chunk(p + 2, rp2qn, rp2kn)
                if p == 1:
                    fillers += [v_chunk(nt, 1) for nt in range(0, 3)]
                elif p == 2:
                    fillers += [v_chunk(nt, 1) for nt in range(3, 6)]

                    def wpj_dma():
                        nc.sync.dma_start(out=wpj, in_=wpp_d[:, :, :])
                    fillers.append((0, wpj_dma))
                elif p == 3:
                    fillers += [v_chunk(nt, 1) for nt in range(6, NT)]
                dr = attention_pair(p, *qk_tiles.pop(p), fillers)
                dbps[p] = dbp_chunk(p, dr)

            # ---------------- output projection ----------------
            # 2-deep chain pipeline: ct 0..6 of the next chains cover dbp(7)
            # and each other's finals
            jobs = [(ch, nt) for ch in range(2) for nt in range(NT) if nt != 0]
            inflight = [(ch, nt, yp) for ch, nt, yp in pre_chains]
            for ch, nt, yp in inflight:
                proj_mid(ch, nt, yp, 5, 7)
            dbp7 = dbps.pop(NPAIR - 1)
            dbp7[1]()
            while inflight:
                proj_fin(*inflight.pop(0))
                if jobs:
                    ch, nt = jobs.pop(0)
                    yp, f = proj_chain_a(ch, nt, 0, 7)
                    f()
                    inflight.append((ch, nt, yp))
    nc.compile()
    return nc


_NC = None


def _get_nc():
    global _NC
    if _NC is None:
        _NC = _build()
    return _NC


def make_in_maps(x, w_qkv, b_qkv, qn_w, kn_w, w_proj, b_proj):
    import ml_dtypes

    bf16 = ml_dtypes.bfloat16
    x = np.asarray(x, np.float32)                          # [B, N, C]
    w_qkv = np.asarray(w_qkv, np.float32)                  # [3C, C]
    w_proj = np.asarray(w_proj, np.float32)                # [C, C]
    # partition-major relayouts (2KB contiguous per partition line)
    xP = np.ascontiguousarray(
        x.reshape(B, N, CT, 128).transpose(0, 3, 2, 1)).astype(bf16)   # [B,128,CT,N]
    wqP = np.ascontiguousarray(
        w_qkv[:2 * C].reshape(2 * NPAIR, 128, CT, 128).transpose(0, 3, 2, 1)
    ).astype(bf16)                                          # [16,128,CT,128] (j-tiles)
    wvP = np.ascontiguousarray(
        w_qkv[2 * C:].reshape(C, CT, 128).transpose(2, 1, 0)).astype(bf16)  # [128,CT,C]
    wpP = np.ascontiguousarray(
        w_proj.reshape(C, CT, 128).transpose(2, 1, 0)).astype(bf16)         # [128,CT,C]
    scale = np.float32(1.0) / np.sqrt(np.float32(D)).astype(np.float32)
    qnkn = (np.asarray(qn_w, np.float32) * np.asarray(kn_w, np.float32) * scale)
    selq = np.zeros((8, 128), np.float32)
    selk = np.zeros((8, 128), np.float32)
    for g in range(4):
        selq[2 * g, 0:64] = 1.0
        selq[2 * g + 1, 64:128] = 1.0
        selk[2 * g, 0:64] = qnkn
        selk[2 * g + 1, 64:128] = qnkn
    return [
        {
            "xP": xP[b],
            "wqP": wqP,
            "wvP": wvP,
            "wpP": wpP,
            "bqkv": np.asarray(b_qkv, np.float32),
            "bproj": np.asarray(b_proj, np.float32),
            "selq": selq,
            "selk": selk,
        }
        for b in range(B)
    ]


def kernel(x, w_qkv, b_qkv, qn_w, kn_w, w_proj, b_proj, **_ignored):
    nc = _get_nc()
    in_maps = make_in_maps(x, w_qkv, b_qkv, qn_w, kn_w, w_proj, b_proj)
    res = run_bass_kernel_spmd(nc, in_maps, core_ids=list(range(NCORES)))
    return np.stack([np.asarray(res.results[b]["y"]) for b in range(B)]).astype(np.float32)


# revision 3
# speedup vs baseline: 1.2261x; 1.2261x over previous
"""Multi-head attention (B=8, N=1024, C=1024, H=16, D=64) with QK RMS-norm,
data-parallel across 8 NeuronCores (one batch element per core).

v4: engines execute their instruction streams in order, so overlap is baked
into emission order. One software-pipelined loop over head pairs; each
pair's 16 (nh, st) attention slots carry the S^T matmul pair (lookahead-1),
the exp, the AV accumulation, plus "filler" chunks popped from a queue
(next pair's q/k projection + RMS chain, V projection tiles, previous
pair's denominator broadcast, tail projection chains). Fillers are ordered
so PE-independent work covers every cross-engine latency chain — PE never
idles >2us, keeping the HAM clock gate at full rate.

Inputs are host-relayouted to partition-major so every DMA line is >=2KB
contiguous. All matmul inputs bf16 (fp32 PSUM accumulation). RMS rsqrt is
computed on the DVE (bit-trick + 2 Newton steps) so ScalarE only ever
loads the exp table set.
"""

import numpy as np

import concourse.bacc as bacc
import concourse.bass as bass
import concourse.tile as tile
from concourse import mybir
from concourse.bass_utils import run_bass_kernel_spmd

F32 = mybir.dt.float32
F32R = mybir.dt.float32r
BF16 = mybir.dt.bfloat16
I32 = mybir.dt.int32
AF = mybir.ActivationFunctionType
OP = mybir.AluOpType

B, N, C = 8, 1024, 1024
H, D = 16, 64
EPS = 1e-6
NCORES = 8
NT = N // 128
CT = C // 128
NPAIR = H // 2
MAGIC = 0x5F3759DF

SLOT_BUDGET = 1000


def _build():
    nc = bacc.Bacc(None, target_bir_lowering=False)

    xP_d = nc.dram_tensor("xP", [128, CT, N], BF16, kind="ExternalInput")
    wqp_d = nc.dram_tensor("wqP", [2 * NPAIR, 128, CT, 128], BF16, kind="ExternalInput")
    wvp_d = nc.dram_tensor("wvP", [128, CT, C], BF16, kind="ExternalInput")
    wpp_d = nc.dram_tensor("wpP", [128, CT, C], BF16, kind="ExternalInput")
    bqkv_d = nc.dram_tensor("bqkv", [3 * C], F32, kind="ExternalInput")
    bproj_d = nc.dram_tensor("bproj", [C], F32, kind="ExternalInput")
    selq_d = nc.dram_tensor("selq", [8, 128], F32, kind="ExternalInput")
    selk_d = nc.dram_tensor("selk", [8, 128], F32, kind="ExternalInput")
    y_d = nc.dram_tensor("y", [N, C], BF16, kind="ExternalOutput")

    with tile.TileContext(nc) as tc:
        with (
            tc.tile_pool(name="long", bufs=1) as lp,
            tc.tile_pool(name="wq", bufs=3) as wq,
            tc.tile_pool(name="qkp", bufs=6) as qkp,
            tc.tile_pool(name="sqp", bufs=2) as sqp,
            tc.tile_pool(name="rpp", bufs=4) as rpp,
            tc.tile_pool(name="rrw", bufs=4) as rrw,
            tc.tile_pool(name="pbp", bufs=4) as pbp,
            tc.tile_pool(name="stp", bufs=4) as stp,
            tc.tile_pool(name="ysp", bufs=3) as ysp,
            tc.tile_pool(name="pp", bufs=2, space="PSUM") as pp,
            tc.tile_pool(name="sps", bufs=2, space="PSUM") as sps,
            tc.tile_pool(name="oo", bufs=1, space="PSUM") as oo,
        ):
            # ---------------- persistent tiles ----------------
            xt = lp.tile([128, CT, N], BF16, tag="xt", name="xt")
            vaug = [lp.tile([128, H, D + 1], BF16, tag=f"va{i}", name=f"va{i}")
                    for i in range(NT)]
            attnT = [lp.tile([128, N], BF16, tag=f"at{i}", name=f"at{i}")
                     for i in range(NPAIR)]
            wv = lp.tile([128, CT, C], BF16, tag="wv", name="wv")
            wpj = lp.tile([128, CT, C], BF16, tag="wpj", name="wpj")
            selq = lp.tile([98, 128], F32R, tag="selq", name="selq")
            selk = lp.tile([98, 128], F32R, tag="selk", name="selk")
            rq_sb = [lp.tile([98, N], F32R, tag=f"rqs{g}", name=f"rqs{g}") for g in range(2)]
            rk_sb = [lp.tile([98, N], F32R, tag=f"rks{g}", name=f"rks{g}") for g in range(2)]
            dv_sb = [lp.tile([98, N], F32R, tag=f"dvs{g}", name=f"dvs{g}") for g in range(2)]
            ones2 = lp.tile([128, 2], BF16, tag="ones2", name="ones2")
            ones1 = lp.tile([1, 128], F32R, tag="ones1", name="ones1")
            magic = lp.tile([128, 32], I32, tag="magic", name="magic")
            bv_bc = lp.tile([128, C], BF16, tag="bvbc", name="bvbc")
            bp_bc = lp.tile([128, C], F32, tag="bpbc", name="bpbc")

            # ---------------- prologue DMAs + constants ----------------
            def load_jt_dma(jt):
                bias_c = wq.tile([128, 1], F32, tag="biasc", name=f"bc{jt}")
                nc.sync.dma_start(out=bias_c,
                                  in_=bqkv_d[jt * 128:(jt + 1) * 128].unsqueeze(1))
                wts = wq.tile([128, CT, 128], BF16, tag="wts", name=f"wts{jt}")
                nc.sync.dma_start(out=wts, in_=wqp_d[jt, :, :, :])
                return bias_c, wts

            # DMA queue drains in emission order: order strictly by first need.
            pre0 = load_jt_dma(0)
            for ct in range(4):
                nc.sync.dma_start(out=xt[:, ct:ct + 1, :], in_=xP_d[:, ct:ct + 1, :])
            pre8 = load_jt_dma(NPAIR)
            for ct in range(4, CT):
                nc.sync.dma_start(out=xt[:, ct:ct + 1, :], in_=xP_d[:, ct:ct + 1, :])
            browv = wq.tile([1, C], F32R, tag="brow", name="browv")
            nc.sync.dma_start(out=browv, in_=bqkv_d[2 * C:3 * C].unsqueeze(0).bitcast(F32R))
            browp = wq.tile([1, C], F32R, tag="brow", name="browp")
            nc.sync.dma_start(out=browp, in_=bproj_d[:].unsqueeze(0).bitcast(F32R))
            nc.sync.dma_start(out=wv[:, :, 0:512], in_=wvp_d[:, :, 0:512])
            for j in range(4):
                nc.sync.dma_start(out=selq[32 * j:32 * j + 2, :],
                                  in_=selq_d[2 * j:2 * j + 2, :].bitcast(F32R))
                nc.sync.dma_start(out=selk[32 * j:32 * j + 2, :],
                                  in_=selk_d[2 * j:2 * j + 2, :].bitcast(F32R))
            nc.sync.dma_start(out=wv[:, :, 512:1024], in_=wvp_d[:, :, 512:1024])

            nc.vector.memset(ones2, 0.0)
            nc.vector.memset(ones2[0:64, 0:1], 1.0)
            nc.vector.memset(ones2[64:128, 1:2], 1.0)
            nc.vector.memset(ones1.bitcast(F32), 1.0)
            nc.vector.memset(magic, MAGIC)
            for ntx in range(NT):
                nc.vector.memset(vaug[ntx][:, :, D:D + 1], 1.0)

            def bias_bc_mms():
                for (brow, btile) in ((browv, bv_bc), (browp, bp_bc)):
                    for half in range(2):
                        hs = slice(half * 512, (half + 1) * 512)
                        bb = pp.tile([128, 512], F32, tag="pp", name=f"bb{half}")
                        nc.tensor.matmul(out=bb[:, :], lhsT=ones1[:, :],
                                         rhs=brow[:, hs], start=True, stop=True)
                        nc.vector.tensor_copy(out=btile[:, hs], in_=bb[:, :])

            # ---------------- chunked helpers ----------------
            def qk_jtile_chunks(jt, qk, sq, rp2, preload=None):
                """3 chunks: [dma + interleaved ct0-3] [ct4-7 + evac/sq] [ssq+rp2].
                nh0/nh1 chains interleave so consecutive matmuls share lhsT."""
                state = {}

                def c_mm(lo):
                    def f():
                        if lo == 0:
                            bias_c, wts = preload if preload else load_jt_dma(jt)
                            state["bias"], state["wts"] = bias_c, wts
                            state["ps0"] = pp.tile([128, 512], F32, tag="pp",
                                                   name=f"qp{jt}0")
                            state["ps1"] = pp.tile([128, 512], F32, tag="pp",
                                                   name=f"qp{jt}1")
                        for ct in range(lo, lo + 4):
                            for nh in range(2):
                                nc.tensor.matmul(
                                    out=state[f"ps{nh}"][:, :],
                                    lhsT=state["wts"][:, ct, :],
                                    rhs=xt[:, ct, nh * 512:(nh + 1) * 512],
                                    start=(ct == 0), stop=(ct == CT - 1),
                                )
                        if lo == 4:
                            for nh in range(2):
                                nsl = slice(nh * 512, (nh + 1) * 512)
                                nc.vector.tensor_scalar(
                                    out=qk[:, nsl], in0=state[f"ps{nh}"][:, :],
                                    scalar1=state["bias"][:, :], scalar2=None, op0=OP.add)
                                nc.vector.tensor_mul(sq[:, nsl], qk[:, nsl], qk[:, nsl])
                    return f

                def c_ssq():
                    def f():
                        for nh in range(2):
                            nsl = slice(nh * 512, (nh + 1) * 512)
                            psum = state[f"ps{nh}"]
                            nc.tensor.matmul(out=psum[0:2, :], lhsT=ones2[:, :],
                                             rhs=sq[:, nsl], start=True, stop=True)
                            nc.vector.tensor_scalar(out=rp2[:, nsl], in0=psum[0:2, :],
                                                    scalar1=1.0 / D, scalar2=EPS,
                                                    op0=OP.mult, op1=OP.add)
                    return f

                return [(1800, c_mm(0)), (1800, c_mm(4)), (500, c_ssq())]

            def rms_chunk(p, rp2q, rp2k):
                def f():
                    g, j = divmod(p, 4)
                    rr = rrw.tile([128, 32], F32, tag="rr", name=f"rr{p}")
                    t0 = rrw.tile([128, 32], I32, tag="t0", name=f"t0{p}")
                    t1 = rrw.tile([128, 32], F32, tag="t1", name=f"t1{p}")
                    nc.sync.dma_start(out=rr[0:64, :], in_=rp2q[:, :])
                    nc.sync.dma_start(out=rr[64:128, :], in_=rp2k[:, :])
                    vi = rr.bitcast(I32)
                    nc.vector.tensor_scalar(out=t0, in0=vi, scalar1=1, scalar2=None,
                                            op0=OP.logical_shift_right)
                    nc.vector.tensor_tensor(out=t0, in0=magic, in1=t0, op=OP.subtract)
                    y = t0.bitcast(F32)
                    for _ in range(2):
                        nc.vector.tensor_mul(t1, y, y)
                        nc.vector.tensor_mul(t1, t1, rr)
                        nc.vector.tensor_scalar(out=t1, in0=t1, scalar1=-0.5,
                                                scalar2=1.5, op0=OP.mult, op1=OP.add)
                        nc.vector.tensor_mul(y, y, t1)
                    nc.sync.dma_start(out=rq_sb[g][32 * j:32 * j + 2, :],
                                      in_=y[0:64, :].bitcast(F32R))
                    nc.sync.dma_start(out=rk_sb[g][32 * j:32 * j + 2, :],
                                      in_=y[64:128, :].bitcast(F32R))
                return [(100, f)]

            def scale_chunks(p, qk_q, qk_k):
                g, j = divmod(p, 4)
                out = []
                for (sel, rsb, qktile) in ((selq, rq_sb[g], qk_q), (selk, rk_sb[g], qk_k)):
                    for nh in range(2):
                        def f(sel=sel, rsb=rsb, qktile=qktile, nh=nh):
                            nsl = slice(nh * 512, (nh + 1) * 512)
                            bc = pp.tile([128, 512], F32, tag="pp", name=f"sc{p}{nh}")
                            nc.tensor.matmul(out=bc[:, :],
                                             lhsT=sel[32 * j:32 * j + 2, :],
                                             rhs=rsb[32 * j:32 * j + 2, nsl],
                                             start=True, stop=True,
                                             tile_position=(32 * j, 0))
                            nc.vector.tensor_tensor(out=qktile[:, nsl], in0=qktile[:, nsl],
                                                    in1=bc[:, :], op=OP.mult)
                        out.append((300, f))
                return out

            def v_chunk(nt, jh):
                def f():
                    vp = pp.tile([128, 512], F32, tag="pp", name=f"vp{nt}{jh}")
                    for ct in range(CT):
                        nc.tensor.matmul(
                            out=vp[:, :],
                            lhsT=xt[:, ct, nt * 128:(nt + 1) * 128],
                            rhs=wv[:, ct, jh * 512:(jh + 1) * 512],
                            start=(ct == 0), stop=(ct == CT - 1),
                        )
                    dst = vaug[nt][:, jh * 8:(jh + 1) * 8, 0:D]
                    vsrc = vp[:, :].rearrange("p (h d) -> p h d", d=D)
                    bsrc = bv_bc[:, jh * 512:(jh + 1) * 512].rearrange("p (h d) -> p h d", d=D)
                    nc.vector.tensor_tensor(out=dst, in0=vsrc, in1=bsrc, op=OP.add)
                return (1750, f)

            def dbp_chunk(p, dr):
                g, j = divmod(p, 4)

                def f():
                    nc.vector.reciprocal(out=dr, in_=dr)
                    nc.sync.dma_start(out=dv_sb[g][32 * j:32 * j + 2, :],
                                      in_=dr[:, :].bitcast(F32R))
                    dbp = sps.tile([128, 1024], F32, tag="sp", name=f"dbp{p}")
                    for nh in range(2):
                        nsl = slice(nh * 512, (nh + 1) * 512)
                        nc.tensor.matmul(out=dbp[:, nsl],
                                         lhsT=selq[32 * j:32 * j + 2, :],
                                         rhs=dv_sb[g][32 * j:32 * j + 2, nsl],
                                         start=True, stop=True, tile_position=(32 * j, 0))
                    nc.vector.tensor_tensor(out=attnT[p][:, :], in0=attnT[p][:, :],
                                            in1=dbp[:, :], op=OP.mult)
                return (500, f)

            def attention_pair(p, qk_q, qk_k, fillers):
                dr = rrw.tile([128, 16], F32, tag="dr", name=f"dr{p}")
                slots = [(nh, st) for nh in range(2) for st in range(NT)]
                psbs = {}

                def emit_S(nh, st):
                    nsl = slice(nh * 512, (nh + 1) * 512)
                    spt = sps.tile([128, 1024], F32, tag="sp", name=f"sp{p}{nh}{st}")
                    nc.tensor.matmul(
                        out=spt[:, 0:512],
                        lhsT=qk_k[0:64, st * 128:(st + 1) * 128],
                        rhs=qk_q[0:64, nsl],
                        start=True, stop=True, tile_position=(0, 0),
                    )
                    nc.tensor.matmul(
                        out=spt[:, 512:1024],
                        lhsT=qk_k[64:128, st * 128:(st + 1) * 128],
                        rhs=qk_q[64:128, nsl],
                        start=True, stop=True, tile_position=(64, 0),
                    )
                    psb = pbp.tile([128, 2, 512], BF16, tag="psb", name=f"pb{p}{nh}{st}")
                    nc.scalar.activation(
                        out=psb, in_=spt[:, :].rearrange("p (h n) -> p h n", h=2),
                        func=AF.Exp)
                    psbs[(nh, st)] = psb

                o_h = None
                for _ in range(2):
                    if fillers:
                        fillers.pop(0)[1]()
                emit_S(0, 0)
                for i, (nh, st) in enumerate(slots):
                    if st == 0:
                        o_h = oo.tile([D + 1, 2, 512], F32, tag="oo", name=f"o{p}{nh}")
                    budget = SLOT_BUDGET
                    while fillers and budget > 0:
                        cost, f = fillers.pop(0)
                        f()
                        budget -= cost
                    if i + 1 < len(slots):
                        emit_S(*slots[i + 1])
                    psb = psbs.pop((nh, st))
                    nc.tensor.matmul(
                        out=o_h[:, 0, :], lhsT=vaug[st][:, 2 * p, :], rhs=psb[:, 0, :],
                        start=(st == 0), stop=(st == NT - 1),
                    )
                    nc.tensor.matmul(
                        out=o_h[:, 1, :], lhsT=vaug[st][:, 2 * p + 1, :], rhs=psb[:, 1, :],
                        start=(st == 0), stop=(st == NT - 1),
                    )
                    if st == NT - 1:
                        nsl = slice(nh * 512, (nh + 1) * 512)
                        nc.vector.tensor_copy(out=attnT[p][0:64, nsl], in_=o_h[0:D, 0, :])
                        nc.vector.tensor_copy(out=attnT[p][64:128, nsl], in_=o_h[0:D, 1, :])
                        stg = stp.tile([1, 2, 512], F32, tag="stg", name=f"stg{p}{nh}")
                        if nh == 1:
                            # pair end: ScalarE is idle (exps drained) — run the
                            # denominator-row copy there so the recip chain and
                            # the oo slot release don't serialize behind the
                            # attnT casts on the DVE
                            nc.scalar.copy(out=stg, in_=o_h[D:D + 1, :, :])
                        else:
                            nc.vector.tensor_copy(out=stg, in_=o_h[D:D + 1, :, :])
                        nc.sync.dma_start(out=dr[32 * nh:32 * nh + 32, :], in_=stg[:, 0, :])
                        nc.sync.dma_start(out=dr[64 + 32 * nh:96 + 32 * nh, :],
                                          in_=stg[:, 1, :])
                while fillers:
                    fillers.pop(0)[1]()
                return dr

            # ---------------- schedule ----------------
            def make_qk(jt):
                qk = qkp.tile([128, N], BF16, tag="qk", name=f"qk{jt}")
                sq = sqp.tile([128, N], BF16, tag="sq", name=f"sq{jt}")
                rp2 = rpp.tile([2, N], F32, tag="rp2", name=f"rp2{jt}")
                return qk, sq, rp2

            qk_q0, sq_q0, rp2q0 = make_qk(0)
            qk_k0, sq_k0, rp2k0 = make_qk(NPAIR)
            for _, f in qk_jtile_chunks(0, qk_q0, sq_q0, rp2q0, preload=pre0):
                f()
            for _, f in qk_jtile_chunks(NPAIR, qk_k0, sq_k0, rp2k0, preload=pre8):
                f()
            bias_bc_mms()
            v_chunk(0, 0)[1]()
            v_chunk(1, 0)[1]()
            rms_chunk(0, rp2q0, rp2k0)[0][1]()
            v_chunk(2, 0)[1]()
            v_chunk(3, 0)[1]()
            for _, f in scale_chunks(0, qk_q0, qk_k0):
                f()
            for nt in range(4, NT):
                v_chunk(nt, 0)[1]()
            qk_q1, sq_q1, rp2q1 = make_qk(1)
            qk_k1, sq_k1, rp2k1 = make_qk(NPAIR + 1)
            for _, f in (qk_jtile_chunks(1, qk_q1, sq_q1, rp2q1)
                         + qk_jtile_chunks(NPAIR + 1, qk_k1, sq_k1, rp2k1)
                         + rms_chunk(1, rp2q1, rp2k1)):
                f()

            # ---------------- output projection pieces ----------------
            def proj_chain_a(ch, nt, lo, hi):
                """accumulate ct in [lo, hi) into a held psum tile."""
                yp = pp.tile([128, 512], F32, tag="pp", name=f"yp{ch}{nt}")

                def f():
                    for ct in range(lo, hi):
                        nc.tensor.matmul(
                            out=yp[:, :],
                            lhsT=attnT[ct][:, nt * 128:(nt + 1) * 128],
                            rhs=wpj[:, ct, ch * 512:(ch + 1) * 512],
                            start=(ct == 0), stop=False,
                        )
                return yp, f

            def proj_mid(ch, nt, yp, lo, hi):
                for ct in range(lo, hi):
                    nc.tensor.matmul(
                        out=yp[:, :],
                        lhsT=attnT[ct][:, nt * 128:(nt + 1) * 128],
                        rhs=wpj[:, ct, ch * 512:(ch + 1) * 512],
                        start=False, stop=False,
                    )

            def proj_fin(ch, nt, yp):
                ct = CT - 1
                nc.tensor.matmul(
                    out=yp[:, :],
                    lhsT=attnT[ct][:, nt * 128:(nt + 1) * 128],
                    rhs=wpj[:, ct, ch * 512:(ch + 1) * 512],
                    start=False, stop=True,
                )
                ysb = ysp.tile([128, 512], BF16, tag="ysb", name=f"ysb{ch}{nt}")
                nc.vector.tensor_tensor(out=ysb, in0=yp[:, :],
                                        in1=bp_bc[:, ch * 512:(ch + 1) * 512], op=OP.add)
                nc.sync.dma_start(
                    out=y_d[nt * 128:(nt + 1) * 128, ch * 512:(ch + 1) * 512],
                    in_=ysb,
                )

            qk_tiles = {0: (qk_q0, qk_k0), 1: (qk_q1, qk_k1)}
            dbps = {}
            pre_chains = []
            for p in range(NPAIR):
                # scale for pair p+1 (its rsqrt finished a pair ago) leads the
                # queue: always-ready boundary bridges for the PE stream
                indep = []
                if p + 1 < NPAIR:
                    indep += scale_chunks(p + 1, *qk_tiles[p + 1])
                else:
                    for ci in range(2):
                        yp, f = proj_chain_a(ci, 0, 0, 5)
                        pre_chains.append((ci, 0, yp))
                        indep.append((1100, f))
                fillers = indep[:2]
                if p - 2 in dbps:
                    fillers.append(dbps.pop(p - 2))
                if p == NPAIR - 1:
                    fillers.append(dbps.pop(p - 1))
                fillers += indep[2:]
                if p + 2 < NPAIR:
                    jq, jk = p + 2, NPAIR + p + 2
                    qk_qn, sq_qn, rp2qn = make_qk(jq)
                    qk_kn, sq_kn, rp2kn = make_qk(jk)
                    qk_tiles[p + 2] = (qk_qn, qk_kn)
                    fillers += qk_jtile_chunks(jq, qk_qn, sq_qn, rp2qn)
                    fillers += qk_jtile_chunks(jk, qk_kn, sq_kn, rp2kn)
                    fillers += rms_chunk(p + 2, rp2qn, rp2kn)
                if p == 1:
                    fillers += [v_chunk(nt, 1) for nt in range(0, 3)]
                elif p == 2:
                    fillers += [v_chunk(nt, 1) for nt in range(3, 6)]

                    def wpj_dma():
                        nc.sync.dma_start(out=wpj, in_=wpp_d[:, :, :])
                    fillers.append((0, wpj_dma))
                elif p == 3:
                    fillers += [v_chunk(nt, 1) for nt in range(6, NT)]
                dr = attention_pair(p, *qk_tiles.pop(p), fillers)
                dbps[p] = dbp_chunk(p, dr)

            # ---------------- output projection ----------------
            # 2-deep chain pipeline: ct 0..6 of the next chains cover dbp(7)
            # and each other's finals
            jobs = [(ch, nt) for ch in range(2) for nt in range(NT) if nt != 0]
            inflight = [(ch, nt, yp) for ch, nt, yp in pre_chains]
            for ch, nt, yp in inflight:
                proj_mid(ch, nt, yp, 5, 7)
            dbp7 = dbps.pop(NPAIR - 1)
            dbp7[1]()
            while inflight:
                proj_fin(*inflight.pop(0))
                if jobs:
                    ch, nt = jobs.pop(0)
                    yp, f = proj_chain_a(ch, nt, 0, 7)
                    f()
                    inflight.append((ch, nt, yp))
    nc.compile()
    return nc


_NC = None


def _get_nc():
    global _NC
    if _NC is None:
        _NC = _build()
    return _NC


def make_in_maps(x, w_qkv, b_qkv, qn_w, kn_w, w_proj, b_proj):
    import ml_dtypes

    bf16 = ml_dtypes.bfloat16
    x = np.asarray(x, np.float32)                          # [B, N, C]
    w_qkv = np.asarray(w_qkv, np.float32)                  # [3C, C]
    w_proj = np.asarray(w_proj, np.float32)                # [C, C]
    # partition-major relayouts (2KB contiguous per partition line)
    xP = np.ascontiguousarray(
        x.reshape(B, N, CT, 128).transpose(0, 3, 2, 1)).astype(bf16)   # [B,128,CT,N]
    wqP = np.ascontiguousarray(
        w_qkv[:2 * C].reshape(2 * NPAIR, 128, CT, 128).transpose(0, 3, 2, 1)
    ).astype(bf16)                                          # [16,128,CT,128] (j-tiles)
    wvP = np.ascontiguousarray(
        w_qkv[2 * C:].reshape(C, CT, 128).transpose(2, 1, 0)).astype(bf16)  # [128,CT,C]
    wpP = np.ascontiguousarray(
        w_proj.reshape(C, CT, 128).transpose(2, 1, 0)).astype(bf16)         # [128,CT,C]
    scale = np.float32(1.0) / np.sqrt(np.float32(D)).astype(np.float32)
    qnkn = (np.asarray(qn_w, np.float32) * np.asarray(kn_w, np.float32) * scale)
    selq = np.zeros((8, 128), np.float32)
    selk = np.zeros((8, 128), np.float32)
    for g in range(4):
        selq[2 * g, 0:64] = 1.0
        selq[2 * g + 1, 64:128] = 1.0
        selk[2 * g, 0:64] = qnkn
        selk[2 * g + 1, 64:128] = qnkn
    return [
        {
            "xP": xP[b],
            "wqP": wqP,
            "wvP": wvP,
            "wpP": wpP,
            "bqkv": np.asarray(b_qkv, np.float32),
            "bproj": np.asarray(b_proj, np.float32),
            "selq": selq,
            "selk": selk,
        }
        for b in range(B)
    ]


def kernel(x, w_qkv, b_qkv, qn_w, kn_w, w_proj, b_proj, **_ignored):
    nc = _get_nc()
    in_maps = make_in_maps(x, w_qkv, b_qkv, qn_w, kn_w, w_proj, b_proj)
    res = run_bass_kernel_spmd(nc, in_maps, core_ids=list(range(NCORES)))
    return np.stack([np.asarray(res.results[b]["y"]) for b in range(B)]).astype(np.float32)
